# revision 19
# baseline (speedup 1.0000x reference)
"""Trainium2 Bass kernel for the Adapt_Layer MoE-routing problem.

Full-input interface: kernel(**inputs) -> np.ndarray [B, D] float32.
Data-parallel over 8 NeuronCores: batch B=16384 sharded 2048/core,
C=8 stacked expert weights replicated.

Math (per batch row x, probs p):
    expert_c = x @ W[c].T + b[c]
    pred     = sum_c p[c] * expert_c = sum_c (p[c]*x) @ W[c].T + p @ b
    s_p      = pred @ pw_w + pw_b    (gate pre-pass via v_c = W[c].T@pw_w)
    s_f      = x @ fw_w + fw_b
    out      = sigmoid(s_p) * pred + sigmoid(s_f) * x

Device dataflow (the key restructure vs a per-expert-PSUM design):
the prob weighting is folded into the STATIONARY operand: featT_sc[c] =
featT * p[b, c] (built on DVE from a GpSimd-replicated p tile). All 8
experts AND the p@b bias then accumulate into a single PSUM bank per
output half, so evacuation is one ACT per (b-tile, half) instead of 8
copy+add pairs. Optionally the first 2*NPAIR k-chunks of the
contraction run as fp8-e4m3 DoubleRow matmuls (2 k-chunks per
instruction) into a separate bank pair, merged at evacuation with a
1/128 scale (W8 = e4m3(128*W)).
"""

import os
import sys
from contextlib import ExitStack

import numpy as np

sys.path.insert(0, "/opt/trn_rl_repo")

import ml_dtypes

import concourse.bass as bass
import concourse.mybir as mybir
import concourse.tile as tile
from concourse import bacc
from concourse.bass_utils import run_bass_kernel_spmd

BF16 = ml_dtypes.bfloat16
FP8 = ml_dtypes.float8_e4m3fn

B, D, C = 16384, 1024, 8
NCORES = 8
P = 128                   # partitions
NBT = 16                  # B-tiles per core
BL = NBT * P              # batch rows per core
KC = D // P               # 8 k-chunks
H = 512                   # output half width (one PSUM bank of fp32)
NPAIR = int(os.environ.get("KERNEL_NPAIR", "3"))  # fp8 DoubleRow k-pairs (0..4)
KB = KC - 2 * NPAIR       # bf16 k-chunks
KOFF = 2 * NPAIR          # first bf16 k-chunk
W8SCALE = 128.0           # host scale on W8; merged back as 1/128 at evac

# Set by the last run when tracing is enabled (KERNEL_TRACE=1)
LAST_EXEC_NS = None
LAST_RESULTS = None


def _install_profile_shim():
    """Enable NTFF profiling under axon: provide the antenv.axon_hooks module
    the boot shim expects, wire the ctypes hook, and keep artifacts local."""
    import types

    import concourse.bass_utils as bu

    bu.upload_artifacts = lambda tmpdir: tmpdir
    try:
        import antenv.axon_hooks  # noqa: F401
        return
    except ImportError:
        pass
    import antenv

    mod = types.ModuleType("antenv.axon_hooks")
    _h = [None]
    mod.set_axon_ntff_profile_hook = lambda h: _h.__setitem__(0, h)
    mod.get_axon_ntff_profile_hook = lambda: _h[0]
    sys.modules["antenv.axon_hooks"] = mod
    antenv.axon_hooks = mod
    try:
        from trn_agent_boot.trn_boot import _ntff_profile_via_ctypes

        hook = _ntff_profile_via_ctypes("/opt/axon/libaxon_pjrt.so")
        if hook is not None:
            mod.set_axon_ntff_profile_hook(hook)
    except Exception as e:  # profiling is best-effort
        print(f"profile shim failed: {e}")


def _dedupe_ldweights(nc) -> int:
    """Drop InstLdweights that reload the exact weights already in the PE
    array (same weights AP as the previous Ldweights, nothing attached)."""
    dropped = 0
    for f in nc.m.functions:
        for blk in f.blocks:
            insts = blk.instructions
            keep = []
            last_sig = None
            for inst in insts:
                tn = type(inst).__name__
                if tn == "InstLdweights":
                    sig = str(inst.ins[0]) + str(getattr(inst, "perf_mode", None))
                    si = inst.sync_info
                    empty = si is None or (not si.on_wait and not si.on_update)
                    if empty and sig == last_sig:
                        dropped += 1
                        continue
                    last_sig = sig
                keep.append(inst)
            if dropped:
                blk.instructions = keep
    return dropped


def _build_graph(pw_b_f: float) -> bass.Bass:
    f32 = mybir.dt.float32
    bf16 = mybir.dt.bfloat16
    fp8e4 = mybir.dt.float8e4
    AF = mybir.ActivationFunctionType
    ALU = mybir.AluOpType
    DR = mybir.MatmulPerfMode.DoubleRow

    nc = bacc.Bacc()
    featT_p = nc.declare_dram_parameter("featT", [D, BL], bf16, isOutput=False)
    feat_p = nc.declare_dram_parameter("feat", [BL, D], bf16, isOutput=False)
    if KB:
        wb_p = nc.declare_dram_parameter("wb", [C * KB * P, D], bf16, isOutput=False)
    if NPAIR:
        w8_p = nc.declare_dram_parameter(
            "w8", [C * NPAIR * P, 2 * D], fp8e4, isOutput=False
        )
    prob_p = nc.declare_dram_parameter("prob", [BL, C], f32, isOutput=False)
    probT_p = nc.declare_dram_parameter("probT", [C, BL], bf16, isOutput=False)
    probT32_p = nc.declare_dram_parameter("probT32", [C, BL], f32, isOutput=False)
    gmat_p = nc.declare_dram_parameter("gmat", [P, KC * 9], bf16, isOutput=False)
    gbeta_p = nc.declare_dram_parameter("gbeta", [1, 9], bf16, isOutput=False)
    bb_p = nc.declare_dram_parameter("bb", [C, D], bf16, isOutput=False)
    out_p = nc.declare_dram_parameter("out", [BL, D], f32, isOutput=True)

    with ExitStack() as ctx:
        tc = ctx.enter_context(tile.TileContext(nc))

        const = ctx.enter_context(tc.tile_pool(name="const", bufs=1))
        psum = ctx.enter_context(tc.tile_pool(name="psum", bufs=1, space="PSUM"))
        prep_pool = ctx.enter_context(tc.tile_pool(name="prepp", bufs=1))
        sc_pool = ctx.enter_context(tc.tile_pool(name="scp", bufs=1))
        feat_pool = ctx.enter_context(tc.tile_pool(name="featp", bufs=2))
        stage_pool = ctx.enter_context(tc.tile_pool(name="stgp", bufs=1))
        gate_pool = ctx.enter_context(tc.tile_pool(name="gatep", bufs=3))

        # ---- resident inputs ----
        # Issue order = consumption order: featT feeds the gate pre-pass and
        # the prescale DVE; w8 feeds the first (fp8) matmul groups; wb is
        # consumed last within each b-tile.
        gmat_sb = const.tile([P, KC * 9], bf16)
        nc.sync.dma_start(gmat_sb[:], gmat_p[:])
        featT_sb = const.tile([P, KC * BL], bf16)          # [:, k*BL : k*BL+BL]
        nc.sync.dma_start(featT_sb[:, 0:BL], featT_p[0:P, :])
        probT_sb = const.tile([C, BL], bf16)
        nc.sync.dma_start(probT_sb[:], probT_p[:])
        bb_sb = const.tile([C, D], bf16)
        nc.sync.dma_start(bb_sb[:], bb_p[:])
        gbeta_sb = const.tile([1, 9], bf16)
        nc.sync.dma_start(gbeta_sb[:], gbeta_p[:])
        prob_all = const.tile([P, NBT * C], f32)           # [:, bt*C+c]
        nc.sync.dma_start(
            prob_all[:].rearrange("p (t c) -> p t c", c=C),
            prob_p.rearrange("(t p) c -> p t c", p=P),
        )
        # rest of featT, one DMA per k-chunk so the gate pre-pass and the
        # prescale DVE unblock progressively as each chunk lands
        for k in range(1, KC):
            nc.sync.dma_start(
                featT_sb[:, k * BL : (k + 1) * BL], featT_p[k * P : (k + 1) * P, :]
            )
        if NPAIR:
            # cols (kp, c): one DMA per kp covers all experts
            w8_sb = const.tile([P, NPAIR * C * 2 * D], fp8e4)
            for kp in range(NPAIR):
                nc.sync.dma_start(
                    w8_sb[:, kp * C * 2 * D : (kp + 1) * C * 2 * D].rearrange(
                        "p (c d) -> p c d", c=C
                    ),
                    w8_p[kp * C * P : (kp + 1) * C * P, :].rearrange(
                        "(c p) d -> p c d", p=P
                    ),
                )
        if KB:
            wb_sb = const.tile([P, C * KB * D], bf16)      # [:, (c*KB+kb)*D : +D]
            for c in range(C):
                nc.sync.dma_start(
                    wb_sb[:, c * KB * D : (c + 1) * KB * D].rearrange(
                        "p (k d) -> p k d", k=KB
                    ),
                    wb_p[c * KB * P : (c + 1) * KB * P, :].rearrange(
                        "(k p) d -> p k d", p=P
                    ),
                )
        ones1 = const.tile([1, P], bf16)
        nc.vector.memset(ones1[:], 1.0)
        pwb_sb = const.tile([P, 1], f32)
        nc.vector.memset(pwb_sb[:], pw_b_f)
        zero_sb = const.tile([P, 1], f32)
        nc.vector.memset(zero_sb[:], 0.0)

        def lhs_feat(k, bt):
            return featT_sb[:, k * BL + bt * P : k * BL + (bt + 1) * P]

        # ---- gate pre-pass: all gate logits in one PSUM bank [P, NBT*9]
        # while the W stream saturates HBM. k-outer so matmuls start as
        # soon as featT[k] lands.
        pred_w_all = const.tile([P, NBT], f32)
        fw_all = const.tile([P, NBT], f32)
        pw8_all = const.tile([P, NBT], f32)
        # One full PSUM bank; a single accumulation group for all 16 b-tiles'
        # 9-col regions: start=True pending-zeroes the WHOLE bank (2KB zero
        # region), each region's first write then overwrites, later writes
        # accumulate. stop only on the very last matmul into the bank.
        pg_all = psum.tile([P, H], f32, tag="pp", name="pg")
        for k in range(KC):
            for bt in range(NBT):
                nc.tensor.matmul(
                    pg_all[:, bt * 9 : (bt + 1) * 9],
                    lhs_feat(k, bt),
                    gmat_sb[:, k * 9 : (k + 1) * 9],
                    start=(k == 0 and bt == 0),
                    stop=False,
                )
        for bt in range(NBT):
            nc.tensor.matmul(
                pg_all[:, bt * 9 : (bt + 1) * 9],
                ones1[:],
                gbeta_sb[:],
                start=False,
                stop=(bt == NBT - 1),
            )
        # batched gate finishing: one strided DVE multiply across all 16
        # b-tiles' 9-col regions, small strided adds for the p-dot, then two
        # [P, NBT] sigmoids. ~2us total so gates never block the f-bank
        # evacuations.
        pg3 = pg_all[:, 0 : NBT * 9].rearrange("p (t n) -> p t n", n=9)
        junk_all = gate_pool.tile([P, NBT * C], f32, name="junk_all")
        nc.vector.tensor_tensor(
            junk_all[:].rearrange("p (t c) -> p t c", c=C),
            pg3[:, :, 0:C],
            prob_all[:].rearrange("p (t c) -> p t c", c=C),
            op=ALU.mult,
        )
        sp_all = gate_pool.tile([P, NBT], f32, name="sp_all")
        j3 = junk_all[:].rearrange("p (t c) -> p t c", c=C)
        nc.vector.tensor_tensor(sp_all[:], j3[:, :, 0], j3[:, :, 1], op=ALU.add)
        for c in range(2, C):
            nc.vector.tensor_tensor(sp_all[:], sp_all[:], j3[:, :, c], op=ALU.add)
        nc.scalar.activation(pred_w_all[:], sp_all[:], AF.Sigmoid, bias=pwb_sb[:])
        nc.scalar.activation(fw_all[:], pg3[:, :, C], AF.Sigmoid, bias=zero_sb[:])
        if NPAIR:
            nc.vector.tensor_scalar_mul(pw8_all[:], pred_w_all[:], 1.0 / W8SCALE)

        # ---- main loop over B-tiles ----
        # PSUM tags: f0/f1 = fp8 banks (single-buffered; evacuated early in
        # the b-tile), b0..b3 = bias+bf16 banks (alternating pairs so the
        # next tile's bias matmul never waits on this tile's evacuation).
        # Startup phase A: the first PH_A tiles run their fp8 groups
        # back-to-back (they need only featT+w8, 8.6MB) so the PE stays fed
        # while the 8MB bf16 W stream is still landing.
        preps_q = {}
        sc8s_all = {}
        scbs_all = {}
        pf_all = {}
        t1s_all = {}
        pb_all = {}

        def emit_prep_quad(q):
            preps = []
            for c in range(C):
                stg = prep_pool.tile([1, 4 * P], f32, name=f"pstg{c}", tag="ps", bufs=2)
                nc.gpsimd.dma_start(
                    stg[:], probT32_p[c : c + 1, q * 4 * P : (q + 1) * 4 * P]
                )
                pr32 = prep_pool.tile([P, 4 * P], f32, name=f"prep32{c}", tag="pr32", bufs=2)
                nc.gpsimd.partition_broadcast(pr32[:], stg[:])
                # bf16 copy (DVE): keeps the prescale multiplies at 2x rate
                pr = prep_pool.tile([P, 4 * P], bf16, name=f"prep{c}", tag=f"pr{c}", bufs=1)
                nc.vector.tensor_copy(pr[:], pr32[:])
                preps.append(pr)
            preps_q[q] = preps

        def prep_slice(bt, c):
            q, qi = divmod(bt, 4)
            return preps_q[q][c][:, qi * P : (qi + 1) * P]

        featT_3d = featT_sb[:].rearrange("p (k b) -> p k b", b=BL)

        def wide_prescale(bt, c, t, k0, nk):
            # one DVE op covering nk k-chunks: featT is BL-strided along k,
            # the p tile broadcasts (stride 0) across the k dim
            in0 = featT_3d[:, k0 : k0 + nk, bt * P : (bt + 1) * P]
            in1 = prep_slice(bt, c).rearrange("p (one m) -> p one m", one=1)
            a, b_ = bass.broadcast_tensor_aps(in0, in1)
            nc.vector.tensor_tensor(
                t[:].rearrange("p (k m) -> p k m", m=P), a, b_, op=ALU.mult
            )

        def emit_sc8(bt):
            sc8s = {}
            for c in range(C):
                t = sc_pool.tile(
                    [P, KOFF * P], fp8e4, name=f"s8_{c}", tag=f"s8_{c}", bufs=2
                )
                wide_prescale(bt, c, t, 0, KOFF)
                sc8s[c] = t
            sc8s_all[bt] = sc8s

        def emit_scb(bt):
            scbs = {}
            for c in range(C):
                t = sc_pool.tile(
                    [P, KB * P], bf16, name=f"sb_{c}", tag=f"sb_{c}", bufs=2
                )
                wide_prescale(bt, c, t, KOFF, KB)
                scbs[c] = t
            scbs_all[bt] = scbs

        def emit_f(bt):
            pf = [psum.tile([P, H], f32, tag=f"f{h}", name=f"pf{h}") for h in range(2)]
            for kp in range(NPAIR):
                for c in range(C):
                    lhs3 = sc8s_all[bt][c][:, kp * 2 * P : (kp + 1) * 2 * P].rearrange(
                        "p (two m) -> p two m", two=2
                    )
                    i = kp * C + c
                    mv = w8_sb[:, i * 2 * D : (i + 1) * 2 * D].rearrange(
                        "p (two hh o) -> p two hh o", two=2, hh=2
                    )
                    for h in range(2):
                        nc.tensor.matmul(
                            pf[h][:],
                            lhs3,
                            mv[:, :, h, :],
                            start=(kp == 0 and c == 0),
                            stop=(kp == NPAIR - 1 and c == C - 1),
                            perf_mode=DR,
                        )
            pf_all[bt] = pf

        def emit_f_evac(bt):
            t1s = []
            for h in range(2):
                t1 = stage_pool.tile([P, H], bf16, name=f"t1{h}", tag=f"t1{h}", bufs=4)
                nc.scalar.activation(
                    t1[:], pf_all[bt][h][:], AF.Copy, scale=pw8_all[:, bt : bt + 1]
                )
                t1s.append(t1)
            t1s_all[bt] = t1s

        def emit_bias_b(bt):
            bp = bt % 2
            pb = [
                psum.tile([P, H], f32, tag=f"b{2 * bp + h}", name=f"pb{h}")
                for h in range(2)
            ]
            lhs_probT = probT_sb[:, bt * P : (bt + 1) * P]
            nc.tensor.matmul(pb[0][:], lhs_probT, bb_sb[:, 0:H], start=True, stop=False)
            nc.tensor.matmul(pb[1][:], lhs_probT, bb_sb[:, H:D], start=True, stop=False)
            for c in range(C):
                for kb in range(KB):
                    lhs = scbs_all[bt][c][:, kb * P : (kb + 1) * P]
                    for h in range(2):
                        nc.tensor.matmul(
                            pb[h][:],
                            lhs,
                            wb_sb[:, (c * KB + kb) * D + h * H : (c * KB + kb) * D + h * H + H],
                            start=False,
                            stop=(c == C - 1 and kb == KB - 1),
                        )
            pb_all[bt] = pb

        def emit_epilogue(bt):
            feat_sb = feat_pool.tile([P, D], bf16)
            nc.sync.dma_start(feat_sb[:], feat_p[bt * P : (bt + 1) * P, :])
            for h in range(2):
                t0 = stage_pool.tile([P, H], f32, name=f"t0{h}", tag=f"t0{h}", bufs=2)
                nc.scalar.activation(
                    t0[:], pb_all[bt][h][:], AF.Copy, scale=pred_w_all[:, bt : bt + 1]
                )
                ft = stage_pool.tile([P, H], bf16, name=f"ft{h}", tag=f"ft{h}", bufs=1)
                nc.scalar.activation(
                    ft[:],
                    feat_sb[:, h * H : (h + 1) * H],
                    AF.Copy,
                    scale=fw_all[:, bt : bt + 1],
                )
                if NPAIR:
                    nc.vector.tensor_tensor(t0[:], t0[:], t1s_all[bt][h][:], op=ALU.add)
                nc.vector.tensor_tensor(t0[:], t0[:], ft[:], op=ALU.add)
                nc.sync.dma_start(
                    out_p[bt * P : (bt + 1) * P, h * H : (h + 1) * H], t0[:]
                )

        PH_A = 3 if NPAIR else 0
        for bt in range(PH_A):
            if bt % 4 == 0:
                emit_prep_quad(bt // 4)
            emit_sc8(bt)
            emit_f(bt)
            emit_f_evac(bt)
        for bt in range(PH_A):
            emit_scb(bt)
            emit_bias_b(bt)
            emit_epilogue(bt)
        for bt in range(PH_A, NBT):
            if bt % 4 == 0:
                emit_prep_quad(bt // 4)
            if NPAIR:
                emit_sc8(bt)
                emit_f(bt)
                emit_f_evac(bt)
            emit_scb(bt)
            emit_bias_b(bt)
            emit_epilogue(bt)

    if os.environ.get("KERNEL_NO_LDW_DEDUPE") != "1":
        _dedupe_ldweights(nc)
    nc.compile()
    return nc


def _host_prep(feature, prob, W, b, pw_w, pw_b_f, fw_b_f, fw_w):
    """Replicated (non-sharded) host-side weight prep."""
    Wt = np.ascontiguousarray(W.transpose(0, 2, 1))          # [C, d, o]
    host = {}
    if KB:
        host["wb"] = np.ascontiguousarray(Wt[:, KOFF * P :, :]).reshape(
            C * KB * P, D
        ).astype(BF16)
    if NPAIR:
        # rows (c, kp, p), cols (ko, o); value e4m3(128 * W[c, o, (2kp+ko)*P+p])
        w8 = Wt[:, : KOFF * P, :].reshape(C, NPAIR, 2, P, D)
        w8 = np.ascontiguousarray(w8.transpose(1, 0, 3, 2, 4)).reshape(
            NPAIR * C * P, 2 * D
        )
        host["w8"] = (w8 * W8SCALE).astype(FP8)
    G = np.einsum("cod,o->dc", W, pw_w)                      # [D, C]: v_c columns
    G9 = np.concatenate([G, fw_w[:, None]], axis=1)          # [D, 9]
    host["gmat"] = np.ascontiguousarray(
        G9.reshape(KC, P, 9).transpose(1, 0, 2).reshape(P, KC * 9)
    ).astype(BF16)
    host["gbeta"] = np.concatenate([b @ pw_w, [fw_b_f]]).reshape(1, 9).astype(BF16)
    host["bb"] = b.astype(BF16)
    return host


def kernel(feature, prob, W, b, pw_w, pw_b, fw_w, fw_b):
    global LAST_EXEC_NS, LAST_RESULTS
    feature = np.asarray(feature, dtype=np.float32)
    prob = np.asarray(prob, dtype=np.float32)
    W = np.asarray(W, dtype=np.float32)
    b = np.asarray(b, dtype=np.float32)
    pw_w = np.asarray(pw_w, dtype=np.float32)
    fw_w = np.asarray(fw_w, dtype=np.float32)
    pw_b_f = float(np.asarray(pw_b).reshape(-1)[0])
    fw_b_f = float(np.asarray(fw_b).reshape(-1)[0])

    host = _host_prep(feature, prob, W, b, pw_w, pw_b_f, fw_b_f, fw_w)

    in_maps = []
    for i in range(NCORES):
        sl = slice(i * BL, (i + 1) * BL)
        m = {
            "featT": np.ascontiguousarray(feature[sl].T).astype(BF16),
            "feat": feature[sl].astype(BF16),
            "prob": np.ascontiguousarray(prob[sl]),
            "probT": np.ascontiguousarray(prob[sl].T).astype(BF16),
            "probT32": np.ascontiguousarray(prob[sl].T),
        }
        m.update(host)
        in_maps.append(m)

    nc = _build_graph(pw_b_f)
    trace = bool(int(os.environ.get("KERNEL_TRACE", "0")))
    if trace:
        _install_profile_shim()
    res = run_bass_kernel_spmd(
        nc, in_maps, core_ids=list(range(NCORES)), trace=trace
    )
    LAST_EXEC_NS = res.exec_time_ns
    LAST_RESULTS = res
    out = np.concatenate([res.results[i]["out"] for i in range(NCORES)], axis=0)
    return np.asarray(out, dtype=np.float32)


# revision 22
# speedup vs baseline: 1.1879x; 1.1879x over previous
"""Trainium2 Bass kernel for the Adapt_Layer MoE-routing problem.

Full-input interface: kernel(**inputs) -> np.ndarray [B, D] float32.
Data-parallel over 8 NeuronCores: batch B=16384 sharded 2048/core,
C=8 stacked expert weights replicated.

Math (per batch row x, probs p):
    expert_c = x @ W[c].T + b[c]
    pred     = sum_c p[c] * expert_c = sum_c (p[c]*x) @ W[c].T + p @ b
    s_p      = pred @ pw_w + pw_b    (gate pre-pass via v_c = W[c].T@pw_w)
    s_f      = x @ fw_w + fw_b
    out      = sigmoid(s_p) * pred + sigmoid(s_f) * x

Device dataflow (the key restructure vs a per-expert-PSUM design):
the prob weighting is folded into the STATIONARY operand: featT_sc[c] =
featT * p[b, c] (built on DVE from a GpSimd-replicated p tile). All 8
experts AND the p@b bias then accumulate into a single PSUM bank per
output half, so evacuation is one ACT per (b-tile, half) instead of 8
copy+add pairs. Optionally the first 2*NPAIR k-chunks of the
contraction run as fp8-e4m3 DoubleRow matmuls (2 k-chunks per
instruction) into a separate bank pair, merged at evacuation with a
1/128 scale (W8 = e4m3(128*W)).
"""

import os
import sys
from contextlib import ExitStack

import numpy as np

sys.path.insert(0, "/opt/trn_rl_repo")

import ml_dtypes

import concourse.bass as bass
import concourse.mybir as mybir
import concourse.tile as tile
from concourse import bacc
from concourse.bass_utils import run_bass_kernel_spmd

BF16 = ml_dtypes.bfloat16
FP8 = ml_dtypes.float8_e4m3fn

B, D, C = 16384, 1024, 8
NCORES = 8
P = 128                   # partitions
NBT = 16                  # B-tiles per core
BL = NBT * P              # batch rows per core
KC = D // P               # 8 k-chunks
H = 512                   # output half width (one PSUM bank of fp32)
NPAIR = int(os.environ.get("KERNEL_NPAIR", "3"))  # fp8 DoubleRow k-pairs (0..4)
KB = KC - 2 * NPAIR       # bf16 k-chunks
KOFF = 2 * NPAIR          # first bf16 k-chunk
W8SCALE = 128.0           # host scale on W8; merged back as 1/128 at evac

# Set by the last run when tracing is enabled (KERNEL_TRACE=1)
LAST_EXEC_NS = None
LAST_RESULTS = None


def _install_profile_shim():
    """Enable NTFF profiling under axon: provide the antenv.axon_hooks module
    the boot shim expects, wire the ctypes hook, and keep artifacts local."""
    import types

    import concourse.bass_utils as bu

    bu.upload_artifacts = lambda tmpdir: tmpdir
    try:
        import antenv.axon_hooks  # noqa: F401
        return
    except ImportError:
        pass
    import antenv

    mod = types.ModuleType("antenv.axon_hooks")
    _h = [None]
    mod.set_axon_ntff_profile_hook = lambda h: _h.__setitem__(0, h)
    mod.get_axon_ntff_profile_hook = lambda: _h[0]
    sys.modules["antenv.axon_hooks"] = mod
    antenv.axon_hooks = mod
    try:
        from trn_agent_boot.trn_boot import _ntff_profile_via_ctypes

        hook = _ntff_profile_via_ctypes("/opt/axon/libaxon_pjrt.so")
        if hook is not None:
            mod.set_axon_ntff_profile_hook(hook)
    except Exception as e:  # profiling is best-effort
        print(f"profile shim failed: {e}")


def _dedupe_ldweights(nc) -> int:
    """Drop InstLdweights that reload the exact weights already in the PE
    array (same weights AP as the previous Ldweights, nothing attached)."""
    dropped = 0
    for f in nc.m.functions:
        for blk in f.blocks:
            insts = blk.instructions
            keep = []
            last_sig = None
            for inst in insts:
                tn = type(inst).__name__
                if tn == "InstLdweights":
                    sig = str(inst.ins[0]) + str(getattr(inst, "perf_mode", None))
                    si = inst.sync_info
                    empty = si is None or (not si.on_wait and not si.on_update)
                    if empty and sig == last_sig:
                        dropped += 1
                        continue
                    last_sig = sig
                keep.append(inst)
            if dropped:
                blk.instructions = keep
    return dropped


def _build_graph(pw_b_f: float) -> bass.Bass:
    f32 = mybir.dt.float32
    bf16 = mybir.dt.bfloat16
    fp8e4 = mybir.dt.float8e4
    AF = mybir.ActivationFunctionType
    ALU = mybir.AluOpType
    DR = mybir.MatmulPerfMode.DoubleRow

    nc = bacc.Bacc()
    featT_p = nc.declare_dram_parameter("featT", [D, BL], bf16, isOutput=False)
    feat_p = nc.declare_dram_parameter("feat", [BL, D], bf16, isOutput=False)
    if KB:
        wb_p = nc.declare_dram_parameter("wb", [C * KB * P, D], bf16, isOutput=False)
    if NPAIR:
        w8_p = nc.declare_dram_parameter(
            "w8", [C * NPAIR * P, 2 * D], fp8e4, isOutput=False
        )
    prob_p = nc.declare_dram_parameter("prob", [BL, C], f32, isOutput=False)
    probT_p = nc.declare_dram_parameter("probT", [C, BL], bf16, isOutput=False)
    probT32_p = nc.declare_dram_parameter("probT32", [C, BL], f32, isOutput=False)
    gmat_p = nc.declare_dram_parameter("gmat", [P, KC * 9], bf16, isOutput=False)
    gbeta_p = nc.declare_dram_parameter("gbeta", [1, 9], bf16, isOutput=False)
    bb_p = nc.declare_dram_parameter("bb", [C, D], bf16, isOutput=False)
    out_p = nc.declare_dram_parameter("out", [BL, D], f32, isOutput=True)

    with ExitStack() as ctx:
        tc = ctx.enter_context(tile.TileContext(nc))

        const = ctx.enter_context(tc.tile_pool(name="const", bufs=1))
        psum = ctx.enter_context(tc.tile_pool(name="psum", bufs=1, space="PSUM"))
        prep_pool = ctx.enter_context(tc.tile_pool(name="prepp", bufs=1))
        sc_pool = ctx.enter_context(tc.tile_pool(name="scp", bufs=1))
        feat_pool = ctx.enter_context(tc.tile_pool(name="featp", bufs=2))
        stage_pool = ctx.enter_context(tc.tile_pool(name="stgp", bufs=1))
        gate_pool = ctx.enter_context(tc.tile_pool(name="gatep", bufs=3))

        # ---- resident inputs ----
        # Issue order = consumption order: featT feeds the gate pre-pass and
        # the prescale DVE; w8 feeds the first (fp8) matmul groups; wb is
        # consumed last within each b-tile.
        gmat_sb = const.tile([P, KC * 9], bf16)
        nc.sync.dma_start(gmat_sb[:], gmat_p[:])
        featT_sb = const.tile([P, KC * BL], bf16)          # [:, k*BL : k*BL+BL]
        nc.sync.dma_start(featT_sb[:, 0:BL], featT_p[0:P, :])
        probT_sb = const.tile([C, BL], bf16)
        nc.sync.dma_start(probT_sb[:], probT_p[:])
        bb_sb = const.tile([C, D], bf16)
        nc.sync.dma_start(bb_sb[:], bb_p[:])
        gbeta_sb = const.tile([1, 9], bf16)
        nc.sync.dma_start(gbeta_sb[:], gbeta_p[:])
        prob_all = const.tile([P, NBT * C], f32)           # [:, bt*C+c]
        nc.sync.dma_start(
            prob_all[:].rearrange("p (t c) -> p t c", c=C),
            prob_p.rearrange("(t p) c -> p t c", p=P),
        )
        # rest of featT, one DMA per k-chunk so the gate pre-pass and the
        # prescale DVE unblock progressively as each chunk lands
        for k in range(1, KC):
            nc.sync.dma_start(
                featT_sb[:, k * BL : (k + 1) * BL], featT_p[k * P : (k + 1) * P, :]
            )
        preps_q = {}
        sc8s_all = {}
        scbs_all = {}
        pf_all = {}
        t1s_all = {}
        pb_all = {}

        def emit_prep_quad(q, dma_eng=None):
            eng = dma_eng if dma_eng is not None else nc.gpsimd
            preps = []
            for c in range(C):
                stg = prep_pool.tile([1, 4 * P], f32, name=f"pstg{c}", tag="ps", bufs=2)
                eng.dma_start(
                    stg[:], probT32_p[c : c + 1, q * 4 * P : (q + 1) * 4 * P]
                )
                pr32 = prep_pool.tile([P, 4 * P], f32, name=f"prep32{c}", tag="pr32", bufs=2)
                nc.gpsimd.partition_broadcast(pr32[:], stg[:])
                # bf16 copy (DVE): keeps the prescale multiplies at 2x rate
                pr = prep_pool.tile([P, 4 * P], bf16, name=f"prep{c}", tag=f"pr{c}", bufs=1)
                nc.vector.tensor_copy(pr[:], pr32[:])
                preps.append(pr)
            preps_q[q] = preps

        def prep_slice(bt, c):
            q, qi = divmod(bt, 4)
            return preps_q[q][c][:, qi * P : (qi + 1) * P]

        # quad 0 staged via the sync queue BEFORE the weight stream: these
        # 2KB transfers must not queue behind 14MB of W or the whole
        # prescale (and every fp8 group) waits ~30us
        emit_prep_quad(0, dma_eng=nc.sync)

        if NPAIR:
            # cols (kp, c): one DMA per kp covers all experts
            w8_sb = const.tile([P, NPAIR * C * 2 * D], fp8e4)
            for kp in range(NPAIR):
                nc.sync.dma_start(
                    w8_sb[:, kp * C * 2 * D : (kp + 1) * C * 2 * D].rearrange(
                        "p (c d) -> p c d", c=C
                    ),
                    w8_p[kp * C * P : (kp + 1) * C * P, :].rearrange(
                        "(c p) d -> p c d", p=P
                    ),
                )
        if KB:
            wb_sb = const.tile([P, C * KB * D], bf16)      # [:, (c*KB+kb)*D : +D]
            for c in range(C):
                nc.sync.dma_start(
                    wb_sb[:, c * KB * D : (c + 1) * KB * D].rearrange(
                        "p (k d) -> p k d", k=KB
                    ),
                    wb_p[c * KB * P : (c + 1) * KB * P, :].rearrange(
                        "(k p) d -> p k d", p=P
                    ),
                )
        ones1 = const.tile([1, P], bf16)
        nc.vector.memset(ones1[:], 1.0)
        pwb_sb = const.tile([P, 1], f32)
        nc.vector.memset(pwb_sb[:], pw_b_f)
        zero_sb = const.tile([P, 1], f32)
        nc.vector.memset(zero_sb[:], 0.0)

        def lhs_feat(k, bt):
            return featT_sb[:, k * BL + bt * P : k * BL + (bt + 1) * P]

        # ---- gate pre-pass: all gate logits in one PSUM bank [P, NBT*9]
        # while the W stream saturates HBM. k-outer so matmuls start as
        # soon as featT[k] lands.
        pred_w_all = const.tile([P, NBT], f32)
        fw_all = const.tile([P, NBT], f32)
        pw8_all = const.tile([P, NBT], f32)
        # One full PSUM bank; a single accumulation group for all 16 b-tiles'
        # 9-col regions: start=True pending-zeroes the WHOLE bank (2KB zero
        # region), each region's first write then overwrites, later writes
        # accumulate. stop only on the very last matmul into the bank.
        pg_all = psum.tile([P, H], f32, tag="pp", name="pg")
        for k in range(KC):
            for bt in range(NBT):
                nc.tensor.matmul(
                    pg_all[:, bt * 9 : (bt + 1) * 9],
                    lhs_feat(k, bt),
                    gmat_sb[:, k * 9 : (k + 1) * 9],
                    start=(k == 0 and bt == 0),
                    stop=False,
                )
        for bt in range(NBT):
            nc.tensor.matmul(
                pg_all[:, bt * 9 : (bt + 1) * 9],
                ones1[:],
                gbeta_sb[:],
                start=False,
                stop=(bt == NBT - 1),
            )
        # batched gate finishing: one strided DVE multiply across all 16
        # b-tiles' 9-col regions, small strided adds for the p-dot, then two
        # [P, NBT] sigmoids. ~2us total so gates never block the f-bank
        # evacuations.
        pg3 = pg_all[:, 0 : NBT * 9].rearrange("p (t n) -> p t n", n=9)
        junk_all = gate_pool.tile([P, NBT * C], f32, name="junk_all")
        nc.vector.tensor_tensor(
            junk_all[:].rearrange("p (t c) -> p t c", c=C),
            pg3[:, :, 0:C],
            prob_all[:].rearrange("p (t c) -> p t c", c=C),
            op=ALU.mult,
        )
        sp_all = gate_pool.tile([P, NBT], f32, name="sp_all")
        j3 = junk_all[:].rearrange("p (t c) -> p t c", c=C)
        nc.vector.tensor_tensor(sp_all[:], j3[:, :, 0], j3[:, :, 1], op=ALU.add)
        for c in range(2, C):
            nc.vector.tensor_tensor(sp_all[:], sp_all[:], j3[:, :, c], op=ALU.add)
        nc.scalar.activation(pred_w_all[:], sp_all[:], AF.Sigmoid, bias=pwb_sb[:])
        nc.scalar.activation(fw_all[:], pg3[:, :, C], AF.Sigmoid, bias=zero_sb[:])
        if NPAIR:
            nc.vector.tensor_scalar_mul(pw8_all[:], pred_w_all[:], 1.0 / W8SCALE)

        # ---- main loop over B-tiles ----
        # PSUM tags: f0/f1 = fp8 banks (single-buffered; evacuated early in
        # the b-tile), b0..b3 = bias+bf16 banks (alternating pairs so the
        # next tile's bias matmul never waits on this tile's evacuation).
        # Startup phase A: the first PH_A tiles run their fp8 groups
        # back-to-back (they need only featT+w8, 8.6MB) so the PE stays fed
        # while the 8MB bf16 W stream is still landing.
        featT_3d = featT_sb[:].rearrange("p (k b) -> p k b", b=BL)

        def wide_prescale(bt, c, t, k0, nk):
            # one DVE op covering nk k-chunks: featT is BL-strided along k,
            # the p tile broadcasts (stride 0) across the k dim
            in0 = featT_3d[:, k0 : k0 + nk, bt * P : (bt + 1) * P]
            in1 = prep_slice(bt, c).rearrange("p (one m) -> p one m", one=1)
            a, b_ = bass.broadcast_tensor_aps(in0, in1)
            nc.vector.tensor_tensor(
                t[:].rearrange("p (k m) -> p k m", m=P), a, b_, op=ALU.mult
            )

        def emit_sc8(bt):
            sc8s = {}
            for c in range(C):
                t = sc_pool.tile(
                    [P, KOFF * P], fp8e4, name=f"s8_{c}", tag=f"s8_{c}", bufs=2
                )
                wide_prescale(bt, c, t, 0, KOFF)
                sc8s[c] = t
            sc8s_all[bt] = sc8s

        def emit_scb(bt):
            scbs = {}
            for c in range(C):
                t = sc_pool.tile(
                    [P, KB * P], bf16, name=f"sb_{c}", tag=f"sb_{c}", bufs=2
                )
                wide_prescale(bt, c, t, KOFF, KB)
                scbs[c] = t
            scbs_all[bt] = scbs

        def emit_f(bt):
            pf = [psum.tile([P, H], f32, tag=f"f{h}", name=f"pf{h}") for h in range(2)]
            for c in range(C):
                for kp in range(NPAIR):
                    lhs3 = sc8s_all[bt][c][:, kp * 2 * P : (kp + 1) * 2 * P].rearrange(
                        "p (two m) -> p two m", two=2
                    )
                    i = kp * C + c
                    mv = w8_sb[:, i * 2 * D : (i + 1) * 2 * D].rearrange(
                        "p (two hh o) -> p two hh o", two=2, hh=2
                    )
                    for h in range(2):
                        nc.tensor.matmul(
                            pf[h][:],
                            lhs3,
                            mv[:, :, h, :],
                            start=(c == 0 and kp == 0),
                            stop=(c == C - 1 and kp == NPAIR - 1),
                            perf_mode=DR,
                        )
            pf_all[bt] = pf

        def emit_f_evac(bt):
            t1s = []
            for h in range(2):
                t1 = stage_pool.tile([P, H], bf16, name=f"t1{h}", tag=f"t1{h}", bufs=4)
                nc.scalar.activation(
                    t1[:], pf_all[bt][h][:], AF.Copy, scale=pw8_all[:, bt : bt + 1]
                )
                t1s.append(t1)
            t1s_all[bt] = t1s

        def emit_bias_b(bt):
            bp = bt % 2
            pb = [
                psum.tile([P, H], f32, tag=f"b{2 * bp + h}", name=f"pb{h}")
                for h in range(2)
            ]
            lhs_probT = probT_sb[:, bt * P : (bt + 1) * P]
            nc.tensor.matmul(pb[0][:], lhs_probT, bb_sb[:, 0:H], start=True, stop=False)
            nc.tensor.matmul(pb[1][:], lhs_probT, bb_sb[:, H:D], start=True, stop=False)
            for c in range(C):
                for kb in range(KB):
                    lhs = scbs_all[bt][c][:, kb * P : (kb + 1) * P]
                    for h in range(2):
                        nc.tensor.matmul(
                            pb[h][:],
                            lhs,
                            wb_sb[:, (c * KB + kb) * D + h * H : (c * KB + kb) * D + h * H + H],
                            start=False,
                            stop=(c == C - 1 and kb == KB - 1),
                        )
            pb_all[bt] = pb

        def emit_epilogue(bt):
            feat_sb = feat_pool.tile([P, D], bf16)
            nc.sync.dma_start(feat_sb[:], feat_p[bt * P : (bt + 1) * P, :])
            for h in range(2):
                t0 = stage_pool.tile([P, H], f32, name=f"t0{h}", tag=f"t0{h}", bufs=2)
                nc.scalar.activation(
                    t0[:], pb_all[bt][h][:], AF.Copy, scale=pred_w_all[:, bt : bt + 1]
                )
                ft = stage_pool.tile([P, H], bf16, name=f"ft{h}", tag=f"ft{h}", bufs=1)
                nc.scalar.activation(
                    ft[:],
                    feat_sb[:, h * H : (h + 1) * H],
                    AF.Copy,
                    scale=fw_all[:, bt : bt + 1],
                )
                if NPAIR:
                    nc.vector.tensor_tensor(t0[:], t0[:], t1s_all[bt][h][:], op=ALU.add)
                nc.vector.tensor_tensor(t0[:], t0[:], ft[:], op=ALU.add)
                nc.sync.dma_start(
                    out_p[bt * P : (bt + 1) * P, h * H : (h + 1) * H], t0[:]
                )

        PH_A = 3 if NPAIR else 0
        for bt in range(PH_A):
            if bt % 4 == 0 and bt > 0:
                emit_prep_quad(bt // 4)
            emit_sc8(bt)
            emit_f(bt)
            emit_f_evac(bt)
        for bt in range(PH_A):
            emit_scb(bt)
            emit_bias_b(bt)
            emit_epilogue(bt)
        for bt in range(PH_A, NBT):
            if bt % 4 == 0 and bt > 0:
                emit_prep_quad(bt // 4)
            if NPAIR:
                emit_sc8(bt)
                emit_f(bt)
                emit_f_evac(bt)
            emit_scb(bt)
            emit_bias_b(bt)
            emit_epilogue(bt)

    if os.environ.get("KERNEL_NO_LDW_DEDUPE") != "1":
        _dedupe_ldweights(nc)
    nc.compile()
    return nc


def _host_prep(feature, prob, W, b, pw_w, pw_b_f, fw_b_f, fw_w):
    """Replicated (non-sharded) host-side weight prep."""
    Wt = np.ascontiguousarray(W.transpose(0, 2, 1))          # [C, d, o]
    host = {}
    if KB:
        host["wb"] = np.ascontiguousarray(Wt[:, KOFF * P :, :]).reshape(
            C * KB * P, D
        ).astype(BF16)
    if NPAIR:
        # rows (c, kp, p), cols (ko, o); value e4m3(128 * W[c, o, (2kp+ko)*P+p])
        w8 = Wt[:, : KOFF * P, :].reshape(C, NPAIR, 2, P, D)
        w8 = np.ascontiguousarray(w8.transpose(1, 0, 3, 2, 4)).reshape(
            NPAIR * C * P, 2 * D
        )
        host["w8"] = (w8 * W8SCALE).astype(FP8)
    G = np.einsum("cod,o->dc", W, pw_w)                      # [D, C]: v_c columns
    G9 = np.concatenate([G, fw_w[:, None]], axis=1)          # [D, 9]
    host["gmat"] = np.ascontiguousarray(
        G9.reshape(KC, P, 9).transpose(1, 0, 2).reshape(P, KC * 9)
    ).astype(BF16)
    host["gbeta"] = np.concatenate([b @ pw_w, [fw_b_f]]).reshape(1, 9).astype(BF16)
    host["bb"] = b.astype(BF16)
    return host


def kernel(feature, prob, W, b, pw_w, pw_b, fw_w, fw_b):
    global LAST_EXEC_NS, LAST_RESULTS
    feature = np.asarray(feature, dtype=np.float32)
    prob = np.asarray(prob, dtype=np.float32)
    W = np.asarray(W, dtype=np.float32)
    b = np.asarray(b, dtype=np.float32)
    pw_w = np.asarray(pw_w, dtype=np.float32)
    fw_w = np.asarray(fw_w, dtype=np.float32)
    pw_b_f = float(np.asarray(pw_b).reshape(-1)[0])
    fw_b_f = float(np.asarray(fw_b).reshape(-1)[0])

    host = _host_prep(feature, prob, W, b, pw_w, pw_b_f, fw_b_f, fw_w)

    in_maps = []
    for i in range(NCORES):
        sl = slice(i * BL, (i + 1) * BL)
        m = {
            "featT": np.ascontiguousarray(feature[sl].T).astype(BF16),
            "feat": feature[sl].astype(BF16),
            "prob": np.ascontiguousarray(prob[sl]),
            "probT": np.ascontiguousarray(prob[sl].T).astype(BF16),
            "probT32": np.ascontiguousarray(prob[sl].T),
        }
        m.update(host)
        in_maps.append(m)

    nc = _build_graph(pw_b_f)
    trace = bool(int(os.environ.get("KERNEL_TRACE", "0")))
    if trace:
        _install_profile_shim()
    res = run_bass_kernel_spmd(
        nc, in_maps, core_ids=list(range(NCORES)), trace=trace
    )
    LAST_EXEC_NS = res.exec_time_ns
    LAST_RESULTS = res
    out = np.concatenate([res.results[i]["out"] for i in range(NCORES)], axis=0)
    return np.asarray(out, dtype=np.float32)
